# revision 8
# baseline (speedup 1.0000x reference)
"""GCN layer on 8 TRN2 NeuronCores — upload-lean version.

out[d,:] = sum_{e: row[e]==d} val[e] * (x[col[e],:] @ W.T) + bias

The per-exec wall time is dominated by a fixed dispatch floor plus a
per-byte cost on uploaded inputs, so this version minimizes bytes shipped
per core:
  - x is sharded (1/C per core, bf16) and AllGathered on-device into an
    internal HBM tensor that the gathers read (12.8MB total vs 102MB
    replicated).
  - gather indices ship compact [16, L/16] and are replicated to the
    [128, L/16] wrapped layout on-device.
  - din ships as uint8, val as bf16; cast to f32 on-device (tensor_scalar
    is_equal requires f32 scalars).
  - output is written bf16.
Compute structure (per core): dest tiles x (lo/hi) gather batches; selector
matrices on DVE; segment-sum via PE matmuls; W + bias per dest tile.
kernel() falls back to a 2-core replicated-x program (no collective) if the
collective path fails to build or run.
"""

import math

import numpy as np
import ml_dtypes

import concourse.bacc as bacc
import concourse.mybir as mybir
import concourse.tile as tile
from concourse.bass_utils import run_bass_kernel_spmd

N_NODES = 50000
N_EDGES = 800000
D = 128
P = 128
SPLIT = 32768

C = 8
G_TILES = 4
CALL = 1024
SINGLE_PACKET = True

BF16 = mybir.dt.bfloat16
F32 = mybir.dt.float32
I16 = mybir.dt.int16
U8 = mybir.dt.uint8


def _ru(x, m):
    return (x + m - 1) // m * m


def _prep_host(edge_row, edge_col, edge_val, n_nodes=N_NODES, c=C):
    er = np.asarray(edge_row).astype(np.int64)
    ec = np.asarray(edge_col).astype(np.int64)
    ev = np.asarray(edge_val).astype(np.float32)

    npc = _ru(math.ceil(n_nodes / c), P)
    t_tiles = npc // P
    core = np.minimum(er // npc, c - 1)
    dloc = er - core * npc
    tl = dloc // P
    din = dloc % P
    half = (ec >= SPLIT).astype(np.int64)

    cnt = np.zeros((c, t_tiles, 2), np.int64)
    np.add.at(cnt, (core, tl, half), 1)

    n_pad = np.zeros((t_tiles, 2), np.int64)
    for t in range(t_tiles):
        for h in range(2):
            n_pad[t, h] = _ru(max(int(cnt[:, t, h].max()), 1), P)

    batches = [list(range(b, min(b + G_TILES, t_tiles)))
               for b in range(0, t_tiles, G_TILES)]
    seg_off = np.zeros((t_tiles, 2), np.int64)
    call_off = []
    off = 0
    for bt in batches:
        lo_off = off
        for t in bt:
            seg_off[t, 0] = off
            off += n_pad[t, 0]
        lo_len = off - lo_off
        hi_off = off
        for t in bt:
            seg_off[t, 1] = off
            off += n_pad[t, 1]
        call_off.append((lo_off, lo_len, hi_off, off - hi_off))
    L = off
    K = L // P

    order = np.lexsort((ec, half, tl, core))
    so = seg_off[tl[order], half[order]]
    key = (core[order] * t_tiles + tl[order]) * 2 + half[order]
    newgrp = np.ones(len(key), bool)
    newgrp[1:] = key[1:] != key[:-1]
    idxs = np.arange(len(key))
    grp_start = np.maximum.accumulate(np.where(newgrp, idxs, 0))
    rank = idxs - grp_start
    pos = so + rank

    idx_flat = np.zeros((c, L), np.int16)
    din_flat = np.full((c, L), 255, np.uint8)   # pad: never equals iota 0..127
    val_flat = np.zeros((c, L), np.float32)
    oc = core[order]
    rebased = np.where(half[order] == 1, ec[order] - SPLIT, ec[order])
    idx_flat[oc, pos] = rebased.astype(np.int16)
    din_flat[oc, pos] = din[order].astype(np.uint8)
    val_flat[oc, pos] = ev[order]

    return dict(
        npc=npc, t_tiles=t_tiles, n_pad=n_pad, batches=batches,
        seg_off=seg_off, call_off=call_off, L=L, K=K, idx_flat=idx_flat,
        din_flat=din_flat, val_flat=val_flat, c=c, n_nodes=n_nodes,
    )


def _build_program(st, use_collective=True):
    n_pad, batches, seg_off, call_off = (
        st["n_pad"], st["batches"], st["seg_off"], st["call_off"])
    L, K, t_tiles, npc = st["L"], st["K"], st["t_tiles"], st["npc"]
    c = st["c"]
    shard_rows = npc                  # x shard rows per core (128-aligned)
    tot_rows = c * shard_rows         # >= n_nodes

    nc = bacc.Bacc("TRN2", target_bir_lowering=False)
    if use_collective:
        xs_d = nc.dram_tensor("xs", [shard_rows, D], BF16,
                              kind="ExternalInput")
    else:
        n_lo = min(SPLIT, tot_rows)
        n_hi = max(tot_rows - SPLIT, P)
        xlo_d = nc.dram_tensor("x_lo", [n_lo, D], BF16, kind="ExternalInput")
        xhi_d = nc.dram_tensor("x_hi", [n_hi, D], BF16, kind="ExternalInput")
    idxc_d = nc.dram_tensor("idxc", [16, L // 16], I16, kind="ExternalInput")
    din_d = nc.dram_tensor("din", [P, K], U8, kind="ExternalInput")
    val_d = nc.dram_tensor("val", [P, K], BF16, kind="ExternalInput")
    wt_d = nc.dram_tensor("wt", [P, D], BF16, kind="ExternalInput")
    iota_d = nc.dram_tensor("iota", [P, P], BF16, kind="ExternalInput")
    bias_d = nc.dram_tensor("bias_row", [1, D], BF16, kind="ExternalInput")
    ones_d = nc.dram_tensor("ones_row", [1, P], BF16, kind="ExternalInput")
    out_d = nc.dram_tensor("out", [npc, D], BF16, kind="ExternalOutput")

    if use_collective:
        xs_i = nc.dram_tensor("xs_i", [shard_rows, D], BF16, kind="Internal")
        x_full = nc.dram_tensor("x_full", [tot_rows, D], BF16,
                                kind="Internal", addr_space="Shared")

    kb_max = max((lo + hi) // P for (_, lo, _, hi) in call_off)

    with tile.TileContext(nc) as tc:
        with (
            tc.tile_pool(name="const", bufs=1) as cpool,
            tc.tile_pool(name="msgs", bufs=2) as mpool,
            tc.tile_pool(name="st", bufs=8) as spool,
            tc.tile_pool(name="aggp", bufs=2, space="PSUM") as agg_pool,
            tc.tile_pool(name="outp", bufs=2, space="PSUM") as outp_pool,
            tc.tile_pool(name="aggs", bufs=3) as aggs_pool,
            tc.tile_pool(name="outs", bufs=3) as outs_pool,
        ):
            if use_collective:
                # stage x shard into internal HBM, AllGather on device
                stage = cpool.tile([P, shard_rows // P, D], BF16)
                nc.gpsimd.dma_start(
                    out=stage[:],
                    in_=xs_d[:].rearrange("(r p) f -> p r f", p=P))
                nc.gpsimd.dma_start(
                    out=xs_i[:].rearrange("(r p) f -> p r f", p=P),
                    in_=stage[:])
                nc.gpsimd.collective_compute(
                    "AllGather",
                    mybir.AluOpType.bypass,
                    replica_groups=[list(range(c))],
                    ins=[xs_i[:].opt()],
                    outs=[x_full[:].opt()],
                )
                x_lo = x_full[0:SPLIT, :] if tot_rows > SPLIT else None
                x_hi = (x_full[SPLIT:tot_rows, :]
                        if tot_rows > SPLIT else x_full[0:tot_rows, :])
            else:
                x_lo = xlo_d[:] if tot_rows > SPLIT else None
                x_hi = xhi_d[:] if tot_rows > SPLIT else xlo_d[:]

            # ---- constants / metadata ----
            idx_sb = cpool.tile([P, L // 16], I16)
            for g in range(8):
                nc.sync.dma_start(out=idx_sb[16 * g:16 * (g + 1), :],
                                  in_=idxc_d[:])
            din8_sb = cpool.tile([P, K], U8)
            val16_sb = cpool.tile([P, K], BF16)
            din_sb = cpool.tile([P, K], F32)
            val_sb = cpool.tile([P, K], F32)
            nc.sync.dma_start(out=din8_sb[:], in_=din_d[:])
            nc.sync.dma_start(out=val16_sb[:], in_=val_d[:])
            nc.scalar.copy(out=din_sb[:], in_=din8_sb[:])
            nc.scalar.copy(out=val_sb[:], in_=val16_sb[:])
            wt_sb = cpool.tile([P, D], BF16)
            iota_sb = cpool.tile([P, P], BF16)
            bias_sb = cpool.tile([1, D], BF16)
            ones_sb = cpool.tile([1, P], BF16)
            nc.sync.dma_start(out=wt_sb[:], in_=wt_d[:])
            nc.sync.dma_start(out=iota_sb[:], in_=iota_d[:])
            nc.sync.dma_start(out=bias_sb[:], in_=bias_d[:])
            nc.sync.dma_start(out=ones_sb[:], in_=ones_d[:])

            def _emit_batch(bi, bt):
                lo_off, lo_len, hi_off, hi_len = call_off[bi]
                boff = lo_off
                msgs = mpool.tile([P, kb_max, D], BF16, tag="msgs")
                for off0, ln, table in ((lo_off, lo_len, x_lo),
                                        (hi_off, hi_len, x_hi)):
                    if table is None:
                        continue
                    for so in range(0, ln, CALL):
                        sl = min(CALL, ln - so)
                        c0 = (off0 + so - boff) // P
                        nc.gpsimd.dma_gather(
                            out_ap=msgs[:, c0:c0 + sl // P, :],
                            in_ap=table,
                            idxs_ap=idx_sb[:, (off0 + so) // 16:
                                           (off0 + so + sl) // 16],
                            num_idxs=sl,
                            num_idxs_reg=sl,
                            elem_size=D,
                            single_packet=SINGLE_PACKET,
                        )
                outs = outs_pool.tile([P, len(bt), D], BF16, tag="outs")
                for ti, t in enumerate(bt):
                    kt = int((n_pad[t, 0] + n_pad[t, 1]) // P)
                    aggp = agg_pool.tile([P, P], F32, tag="aggp")
                    j = 0
                    for h in range(2):
                        g0 = int(seg_off[t, h]) // P
                        c0 = (int(seg_off[t, h]) - boff) // P
                        for q in range(int(n_pad[t, h]) // P):
                            stile = spool.tile([P, P], BF16, tag="st")
                            nc.vector.tensor_scalar(
                                out=stile[:],
                                in0=iota_sb[:],
                                scalar1=din_sb[:, g0 + q:g0 + q + 1],
                                scalar2=val_sb[:, g0 + q:g0 + q + 1],
                                op0=mybir.AluOpType.is_equal,
                                op1=mybir.AluOpType.mult,
                            )
                            nc.tensor.matmul(
                                out=aggp[:],
                                lhsT=msgs[:, c0 + q, :],
                                rhs=stile[:],
                                start=(j == 0),
                                stop=(j == kt - 1),
                            )
                            j += 1
                    aggs = aggs_pool.tile([P, P], BF16, tag="aggs")
                    nc.scalar.copy(out=aggs[:], in_=aggp[:])
                    outp = outp_pool.tile([P, D], F32, tag="outp")
                    nc.tensor.matmul(out=outp[:], lhsT=aggs[:], rhs=wt_sb[:],
                                     start=True, stop=False)
                    nc.tensor.matmul(out=outp[:], lhsT=ones_sb[:],
                                     rhs=bias_sb[:], start=False, stop=True)
                    nc.scalar.copy(out=outs[:, ti, :], in_=outp[:])
                r0 = bt[0] * P
                rows = (bt[-1] + 1) * P - r0
                hbm = out_d[r0:r0 + rows, :].rearrange("(c p) f -> p c f", p=P)
                nc.sync.dma_start(out=hbm, in_=outs[:, :rows // P, :])

            for bi, bt in enumerate(batches):
                _emit_batch(bi, bt)
    nc.compile()
    return nc


def _compact_idx(idx_flat_core):
    L = idx_flat_core.shape[0]
    return np.ascontiguousarray(idx_flat_core.reshape(L // 16, 16).T)


def make_in_maps(x, W, bias, st, use_collective=True):
    c, npc = st["c"], st["npc"]
    x32 = np.asarray(x, np.float32)
    x_pad = np.zeros((c * npc, D), np.float32)
    x_pad[:x32.shape[0]] = x32
    x_bf = x_pad.astype(ml_dtypes.bfloat16)
    wt = np.ascontiguousarray(np.asarray(W, np.float32).T).astype(
        ml_dtypes.bfloat16)
    iota = np.tile(np.arange(P, dtype=np.float32), (P, 1)).astype(
        ml_dtypes.bfloat16)
    bias_row = np.asarray(bias, np.float32)[None, :].astype(ml_dtypes.bfloat16)
    ones_row = np.ones((1, P), ml_dtypes.bfloat16)

    K = st["K"]
    din_cols = st["din_flat"].reshape(c, K, P).transpose(0, 2, 1)
    val_cols = st["val_flat"].reshape(c, K, P).transpose(0, 2, 1)

    in_maps = []
    for ci in range(c):
        m = {
            "idxc": _compact_idx(st["idx_flat"][ci]),
            "din": np.ascontiguousarray(din_cols[ci]),
            "val": np.ascontiguousarray(val_cols[ci]).astype(
                ml_dtypes.bfloat16),
            "wt": wt, "iota": iota, "bias_row": bias_row,
            "ones_row": ones_row,
        }
        if use_collective:
            m["xs"] = np.ascontiguousarray(x_bf[ci * npc:(ci + 1) * npc])
        else:
            tot = c * npc
            m["x_lo"] = np.ascontiguousarray(x_bf[:min(SPLIT, tot)])
            m["x_hi"] = (np.ascontiguousarray(x_bf[SPLIT:]) if tot > SPLIT
                         else np.zeros((P, D), ml_dtypes.bfloat16))
        in_maps.append(m)
    return in_maps


def _run(x, edge_row, edge_col, edge_val, W, bias, c, use_collective):
    st = _prep_host(edge_row, edge_col, edge_val, c=c)
    nc = _build_program(st, use_collective=use_collective)
    in_maps = make_in_maps(x, W, bias, st, use_collective=use_collective)
    res = run_bass_kernel_spmd(nc, in_maps, core_ids=list(range(c)))
    out = np.concatenate([res.results[ci]["out"] for ci in range(c)], axis=0)
    return out[:N_NODES].astype(np.float32)


def kernel(x, edge_row, edge_col, edge_val, W, bias):
    try:
        return _run(x, edge_row, edge_col, edge_val, W, bias,
                    c=C, use_collective=True)
    except Exception:
        # fallback: replicated x on 2 cores, no collective
        return _run(x, edge_row, edge_col, edge_val, W, bias,
                    c=2, use_collective=False)


if __name__ == "__main__":
    rng = np.random.default_rng(0)
    x = rng.standard_normal((N_NODES, D), dtype=np.float32)
    er = rng.integers(0, N_NODES, N_EDGES)
    ec = rng.integers(0, N_NODES, N_EDGES)
    ev = rng.random(N_EDGES, dtype=np.float32)
    W = rng.standard_normal((D, D), dtype=np.float32) / np.sqrt(D)
    b = np.zeros(D, np.float32)
    out = kernel(x, er, ec, ev, W, b)
    print(out.shape, out.dtype)


# revision 13
# speedup vs baseline: 1.0050x; 1.0050x over previous
"""GCN layer on 8 TRN2 NeuronCores — upload-lean version.

out[d,:] = sum_{e: row[e]==d} val[e] * (x[col[e],:] @ W.T) + bias

The per-exec wall time is dominated by a fixed dispatch floor plus a
per-byte cost on uploaded inputs, so this version minimizes bytes shipped
per core:
  - x is sharded (1/C per core, bf16) and AllGathered on-device into an
    internal HBM tensor that the gathers read (12.8MB total vs 102MB
    replicated).
  - gather indices ship compact [16, L/16] and are replicated to the
    [128, L/16] wrapped layout on-device.
  - din ships as uint8, val as bf16; cast to f32 on-device (tensor_scalar
    is_equal requires f32 scalars).
  - output is written bf16.
Compute structure (per core): dest tiles x (lo/hi) gather batches; selector
matrices on DVE; segment-sum via PE matmuls; W + bias per dest tile.
kernel() falls back to a 2-core replicated-x program (no collective) if the
collective path fails to build or run.
"""

import math

import numpy as np
import ml_dtypes

import concourse.bacc as bacc
import concourse.mybir as mybir
import concourse.tile as tile
from concourse.bass_utils import run_bass_kernel_spmd

N_NODES = 50000
N_EDGES = 800000
D = 128
P = 128
SPLIT = 32768

C = 8
G_TILES = 4
CALL = 1024
SINGLE_PACKET = True

BF16 = mybir.dt.bfloat16
F32 = mybir.dt.float32
I16 = mybir.dt.int16
U8 = mybir.dt.uint8


def _ru(x, m):
    return (x + m - 1) // m * m


def _prep_host(edge_row, edge_col, edge_val, n_nodes=N_NODES, c=C):
    er = np.asarray(edge_row).astype(np.int64)
    ec = np.asarray(edge_col).astype(np.int64)
    ev = np.asarray(edge_val).astype(np.float32)

    npc = _ru(math.ceil(n_nodes / c), P)
    t_tiles = npc // P
    core = np.minimum(er // npc, c - 1)
    dloc = er - core * npc
    tl = dloc // P
    din = dloc % P
    half = (ec >= SPLIT).astype(np.int64)

    cnt = np.zeros((c, t_tiles, 2), np.int64)
    np.add.at(cnt, (core, tl, half), 1)

    n_pad = np.zeros((t_tiles, 2), np.int64)
    for t in range(t_tiles):
        for h in range(2):
            n_pad[t, h] = _ru(max(int(cnt[:, t, h].max()), 1), P)

    batches = [list(range(b, min(b + G_TILES, t_tiles)))
               for b in range(0, t_tiles, G_TILES)]
    seg_off = np.zeros((t_tiles, 2), np.int64)
    call_off = []
    off = 0
    for bt in batches:
        lo_off = off
        for t in bt:
            seg_off[t, 0] = off
            off += n_pad[t, 0]
        lo_len = off - lo_off
        hi_off = off
        for t in bt:
            seg_off[t, 1] = off
            off += n_pad[t, 1]
        call_off.append((lo_off, lo_len, hi_off, off - hi_off))
    L = off
    K = L // P

    order = np.lexsort((ec, half, tl, core))
    so = seg_off[tl[order], half[order]]
    key = (core[order] * t_tiles + tl[order]) * 2 + half[order]
    newgrp = np.ones(len(key), bool)
    newgrp[1:] = key[1:] != key[:-1]
    idxs = np.arange(len(key))
    grp_start = np.maximum.accumulate(np.where(newgrp, idxs, 0))
    rank = idxs - grp_start
    pos = so + rank

    idx_flat = np.zeros((c, L), np.int16)
    din_flat = np.full((c, L), 255, np.uint8)   # pad: never equals iota 0..127
    val_flat = np.zeros((c, L), np.float32)
    oc = core[order]
    rebased = np.where(half[order] == 1, ec[order] - SPLIT, ec[order])
    idx_flat[oc, pos] = rebased.astype(np.int16)
    din_flat[oc, pos] = din[order].astype(np.uint8)
    val_flat[oc, pos] = ev[order]

    return dict(
        npc=npc, t_tiles=t_tiles, n_pad=n_pad, batches=batches,
        seg_off=seg_off, call_off=call_off, L=L, K=K, idx_flat=idx_flat,
        din_flat=din_flat, val_flat=val_flat, c=c, n_nodes=n_nodes,
    )


def _build_program(st, use_collective=True):
    n_pad, batches, seg_off, call_off = (
        st["n_pad"], st["batches"], st["seg_off"], st["call_off"])
    L, K, t_tiles, npc = st["L"], st["K"], st["t_tiles"], st["npc"]
    c = st["c"]
    shard_rows = npc                  # x shard rows per core (128-aligned)
    tot_rows = c * shard_rows         # >= n_nodes

    nc = bacc.Bacc("TRN2", target_bir_lowering=False)
    if use_collective:
        xs_d = nc.dram_tensor("xs", [shard_rows, D], BF16,
                              kind="ExternalInput")
    else:
        n_lo = min(SPLIT, tot_rows)
        n_hi = max(tot_rows - SPLIT, P)
        xlo_d = nc.dram_tensor("x_lo", [n_lo, D], BF16, kind="ExternalInput")
        xhi_d = nc.dram_tensor("x_hi", [n_hi, D], BF16, kind="ExternalInput")
    idxc_d = nc.dram_tensor("idxc", [16, L // 16], I16, kind="ExternalInput")
    kp2 = _ru(K, 2) // 2     # din bytes packed as bf16 columns
    # meta (raw bits as int16): cols [0:K) = val bf16, [K:K+kp2) = din u8
    meta_d = nc.dram_tensor("meta", [P, K + kp2], I16, kind="ExternalInput")
    # cst: cols [0:128) = W^T, row 0 cols [128:256) = bias
    cst_d = nc.dram_tensor("cst", [P, 2 * P], BF16, kind="ExternalInput")
    out_d = nc.dram_tensor("out", [npc, D], BF16, kind="ExternalOutput")

    if use_collective:
        xs_i = nc.dram_tensor("xs_i", [shard_rows, D], BF16, kind="Internal")
        x_full = nc.dram_tensor("x_full", [tot_rows, D], BF16,
                                kind="Internal", addr_space="Shared")

    kb_max = max((lo + hi) // P for (_, lo, _, hi) in call_off)

    with tile.TileContext(nc) as tc:
        with (
            tc.tile_pool(name="const", bufs=1) as cpool,
            tc.tile_pool(name="msgs", bufs=2) as mpool,
            tc.tile_pool(name="st", bufs=8) as spool,
            tc.tile_pool(name="aggp", bufs=2, space="PSUM") as agg_pool,
            tc.tile_pool(name="outp", bufs=2, space="PSUM") as outp_pool,
            tc.tile_pool(name="aggs", bufs=3) as aggs_pool,
            tc.tile_pool(name="outs", bufs=3) as outs_pool,
        ):
            if use_collective:
                # stage x shard into internal HBM, AllGather on device
                stage = cpool.tile([P, shard_rows // P, D], BF16)
                nc.gpsimd.dma_start(
                    out=stage[:],
                    in_=xs_d[:].rearrange("(r p) f -> p r f", p=P))
                nc.gpsimd.dma_start(
                    out=xs_i[:].rearrange("(r p) f -> p r f", p=P),
                    in_=stage[:])
                nc.gpsimd.collective_compute(
                    "AllGather",
                    mybir.AluOpType.bypass,
                    replica_groups=[list(range(c))],
                    ins=[xs_i[:].opt()],
                    outs=[x_full[:].opt()],
                )
                x_lo = x_full[0:SPLIT, :] if tot_rows > SPLIT else None
                x_hi = (x_full[SPLIT:tot_rows, :]
                        if tot_rows > SPLIT else x_full[0:tot_rows, :])
            else:
                x_lo = xlo_d[:] if tot_rows > SPLIT else None
                x_hi = xhi_d[:] if tot_rows > SPLIT else xlo_d[:]

            # ---- constants / metadata ----
            idx_sb = cpool.tile([P, L // 16], I16)
            for g in range(8):
                nc.sync.dma_start(out=idx_sb[16 * g:16 * (g + 1), :],
                                  in_=idxc_d[:])
            meta_sb = cpool.tile([P, K + kp2], I16)
            din_sb = cpool.tile([P, K], F32)
            val_sb = cpool.tile([P, K], F32)
            nc.sync.dma_start(out=meta_sb[:], in_=meta_d[:])
            nc.scalar.copy(out=val_sb[:], in_=meta_sb[:, :K].bitcast(BF16))
            nc.scalar.copy(
                out=din_sb[:],
                in_=meta_sb[:, K:K + kp2].bitcast(U8)[:, :K])
            cst_sb = cpool.tile([P, 2 * P], BF16)
            nc.sync.dma_start(out=cst_sb[:], in_=cst_d[:])
            wt_sb = cst_sb[:, :P]
            bias_sb = cst_sb[0:1, P:2 * P]
            iota_sb = cpool.tile([P, P], BF16)
            nc.gpsimd.iota(iota_sb[:], pattern=[[1, P]], base=0,
                           channel_multiplier=0,
                           allow_small_or_imprecise_dtypes=True)
            ones_sb = cpool.tile([1, P], BF16)
            nc.vector.memset(ones_sb[:], 1.0)

            def _emit_batch(bi, bt):
                lo_off, lo_len, hi_off, hi_len = call_off[bi]
                boff = lo_off
                msgs = mpool.tile([P, kb_max, D], BF16, tag="msgs")
                for off0, ln, table in ((lo_off, lo_len, x_lo),
                                        (hi_off, hi_len, x_hi)):
                    if table is None:
                        continue
                    for so in range(0, ln, CALL):
                        sl = min(CALL, ln - so)
                        c0 = (off0 + so - boff) // P
                        nc.gpsimd.dma_gather(
                            out_ap=msgs[:, c0:c0 + sl // P, :],
                            in_ap=table,
                            idxs_ap=idx_sb[:, (off0 + so) // 16:
                                           (off0 + so + sl) // 16],
                            num_idxs=sl,
                            num_idxs_reg=sl,
                            elem_size=D,
                            single_packet=SINGLE_PACKET,
                        )
                outs = outs_pool.tile([P, len(bt), D], BF16, tag="outs")
                for ti, t in enumerate(bt):
                    kt = int((n_pad[t, 0] + n_pad[t, 1]) // P)
                    aggp = agg_pool.tile([P, P], F32, tag="aggp")
                    j = 0
                    for h in range(2):
                        g0 = int(seg_off[t, h]) // P
                        c0 = (int(seg_off[t, h]) - boff) // P
                        for q in range(int(n_pad[t, h]) // P):
                            stile = spool.tile([P, P], BF16, tag="st")
                            nc.vector.tensor_scalar(
                                out=stile[:],
                                in0=iota_sb[:],
                                scalar1=din_sb[:, g0 + q:g0 + q + 1],
                                scalar2=val_sb[:, g0 + q:g0 + q + 1],
                                op0=mybir.AluOpType.is_equal,
                                op1=mybir.AluOpType.mult,
                            )
                            nc.tensor.matmul(
                                out=aggp[:],
                                lhsT=msgs[:, c0 + q, :],
                                rhs=stile[:],
                                start=(j == 0),
                                stop=(j == kt - 1),
                            )
                            j += 1
                    aggs = aggs_pool.tile([P, P], BF16, tag="aggs")
                    nc.scalar.copy(out=aggs[:], in_=aggp[:])
                    outp = outp_pool.tile([P, D], F32, tag="outp")
                    nc.tensor.matmul(out=outp[:], lhsT=aggs[:], rhs=wt_sb,
                                     start=True, stop=False)
                    nc.tensor.matmul(out=outp[:], lhsT=ones_sb[:],
                                     rhs=bias_sb, start=False, stop=True)
                    nc.scalar.copy(out=outs[:, ti, :], in_=outp[:])
                r0 = bt[0] * P
                rows = (bt[-1] + 1) * P - r0
                hbm = out_d[r0:r0 + rows, :].rearrange("(c p) f -> p c f", p=P)
                nc.sync.dma_start(out=hbm, in_=outs[:, :rows // P, :])

            for bi, bt in enumerate(batches):
                _emit_batch(bi, bt)
    nc.compile()
    return nc


def _compact_idx(idx_flat_core):
    L = idx_flat_core.shape[0]
    return np.ascontiguousarray(idx_flat_core.reshape(L // 16, 16).T)


def make_in_maps(x, W, bias, st, use_collective=True):
    c, npc = st["c"], st["npc"]
    x32 = np.asarray(x, np.float32)
    x_pad = np.zeros((c * npc, D), np.float32)
    x_pad[:x32.shape[0]] = x32
    x_bf = x_pad.astype(ml_dtypes.bfloat16)
    wt = np.ascontiguousarray(np.asarray(W, np.float32).T).astype(
        ml_dtypes.bfloat16)
    cst = np.zeros((P, 2 * P), ml_dtypes.bfloat16)
    cst[:, :P] = wt
    cst[0, P:2 * P] = np.asarray(bias, np.float32).astype(ml_dtypes.bfloat16)

    K = st["K"]
    kp2 = _ru(K, 2) // 2
    din_cols = st["din_flat"].reshape(c, K, P).transpose(0, 2, 1)
    val_cols = st["val_flat"].reshape(c, K, P).transpose(0, 2, 1)

    in_maps = []
    for ci in range(c):
        meta = np.zeros((P, K + kp2), np.int16)
        meta[:, :K] = np.ascontiguousarray(val_cols[ci]).astype(
            ml_dtypes.bfloat16).view(np.int16)
        dinb = np.zeros((P, 2 * kp2), np.uint8)
        dinb[:, :K] = din_cols[ci]
        meta[:, K:K + kp2] = dinb.view(np.int16)
        m = {
            "idxc": _compact_idx(st["idx_flat"][ci]),
            "meta": meta,
            "cst": cst,
        }
        if use_collective:
            m["xs"] = np.ascontiguousarray(x_bf[ci * npc:(ci + 1) * npc])
        else:
            tot = c * npc
            m["x_lo"] = np.ascontiguousarray(x_bf[:min(SPLIT, tot)])
            m["x_hi"] = (np.ascontiguousarray(x_bf[SPLIT:]) if tot > SPLIT
                         else np.zeros((P, D), ml_dtypes.bfloat16))
        in_maps.append(m)
    return in_maps


def _run(x, edge_row, edge_col, edge_val, W, bias, c, use_collective):
    st = _prep_host(edge_row, edge_col, edge_val, c=c)
    nc = _build_program(st, use_collective=use_collective)
    in_maps = make_in_maps(x, W, bias, st, use_collective=use_collective)
    res = run_bass_kernel_spmd(nc, in_maps, core_ids=list(range(c)))
    out = np.concatenate([res.results[ci]["out"] for ci in range(c)], axis=0)
    return out[:N_NODES].astype(np.float32)


def kernel(x, edge_row, edge_col, edge_val, W, bias):
    try:
        return _run(x, edge_row, edge_col, edge_val, W, bias,
                    c=C, use_collective=True)
    except Exception:
        # fallback: replicated x on 2 cores, no collective
        return _run(x, edge_row, edge_col, edge_val, W, bias,
                    c=2, use_collective=False)


if __name__ == "__main__":
    rng = np.random.default_rng(0)
    x = rng.standard_normal((N_NODES, D), dtype=np.float32)
    er = rng.integers(0, N_NODES, N_EDGES)
    ec = rng.integers(0, N_NODES, N_EDGES)
    ev = rng.random(N_EDGES, dtype=np.float32)
    W = rng.standard_normal((D, D), dtype=np.float32) / np.sqrt(D)
    b = np.zeros(D, np.float32)
    out = kernel(x, er, ec, ev, W, b)
    print(out.shape, out.dtype)
